# revision 1
# baseline (speedup 1.0000x reference)
"""Self-attention kernel for TRN2: out = softmax(X Wq (X Wk)^T / sqrt(D)) @ X.

Strategy (8-way sequence parallelism over query rows):
  scores = (X Wq)(X Wk)^T = X M X^T  with  M = (Wq/sqrt(D)) Wk^T
so K is never materialized. Each core i handles query rows [i*B, (i+1)*B):
  phase 0: M = Wqs Wk^T (fp32), A_i^T = M^T X_i^T (fp32), both on-device.
  flash:   stream key blocks j; S^T_j = (X^T_j)^T-block logits in key-major
           layout via fp32 matmuls; running column-max via PE transpose +
           reduce; E = exp(S - max) rounded to float32r in-place; second
           matmul E^T-slices @ X_aug in float32r (2x fp32 rate; precision
           is ample there); a ones-column in X_aug accumulates the softmax
           denominator through the same PSUM/rescale path; fused
           rescale-accumulate (acc = acc*corr + psum) on DVE; final divide.

Numerics: logits need ~fp32 precision (std ~1024, near-tie rows amplify
errors through softmax), so everything feeding the logits is fp32 (PE fp32
= 2 cyc/row). The P@X matmul only needs ~1e-4 relative, so float32r
(1 cyc/row) is safe there.
"""
import numpy as np
from contextlib import ExitStack

import concourse.bass as bass
import concourse.bacc as bacc
import concourse.tile as tile
from concourse import mybir
from concourse.bass_utils import run_bass_kernel_spmd
from concourse.masks import make_identity

P = 128
SEQ = 8192
DIM = 1024
NCORES = 8
AUG = 4      # extra columns on X_aug: [ones, 0, 0, 0]
SBN = 4      # key n-tiles (of 128) per flash super-block

F32 = mybir.dt.float32
F32R = mybir.dt.float32r
EXP = mybir.ActivationFunctionType.Exp
ALU = mybir.AluOpType
AXX = mybir.AxisListType.X


def _chunks(total, step=512):
    return [(lo, min(lo + step, total)) for lo in range(0, total, step)]


def build_core_kernel(S, D, B, sbn=SBN, aug=AUG):
    """One core's kernel: query rows block of size B, full S keys."""
    KT = D // P      # contraction tiles over D
    NT = S // P      # key tiles
    MT = B // P      # query tiles (per core)
    NSB = NT // sbn  # super-blocks
    XAW = D + aug
    assert NT % sbn == 0 and B % P == 0 and D % P == 0 and MT <= P

    nc = bacc.Bacc("TRN2", target_bir_lowering=False, debug=False)
    xtj = nc.dram_tensor("xtj", [NT, P, D], F32, kind="ExternalInput")
    xa = nc.dram_tensor("xa", [S, XAW], F32, kind="ExternalInput")
    wqst = nc.dram_tensor("wqst", [D, D], F32, kind="ExternalInput")
    wkt = nc.dram_tensor("wkt", [D, D], F32, kind="ExternalInput")
    xit = nc.dram_tensor("xit", [D, B], F32, kind="ExternalInput")
    out = nc.dram_tensor("out", [B, D], F32, kind="ExternalOutput")

    with tile.TileContext(nc) as tc, ExitStack() as ctx:
        pers = ctx.enter_context(tc.tile_pool(name="pers", bufs=1))
        aith = [pers.tile([P, B], F32R, name=f"aith{k}") for k in range(KT)]
        aitl = [pers.tile([P, B], F32R, name=f"aitl{k}") for k in range(KT)]
        acc = [pers.tile([P, XAW], F32, name=f"acc{t}") for t in range(MT)]
        gm = pers.tile([P, B], F32, name="gm")
        mxbc = pers.tile([P, B], F32, name="mxbc")
        ident = pers.tile([P, P], F32, name="ident")
        make_identity(nc, ident[:])
        for t in range(MT):
            nc.gpsimd.memset(acc[t][:], 0.0)
        nc.gpsimd.memset(gm[:], -1e30)

        # ---- phase 0: M = Wqs Wk^T ; A_i^T = M^T X_i^T ----
        with ExitStack() as p0:
            mpool = p0.enter_context(tc.tile_pool(name="mpool", bufs=1))
            ps0 = p0.enter_context(tc.tile_pool(name="ps0", bufs=4, space="PSUM"))
            m_t = [mpool.tile([P, D], F32, name=f"m{e}") for e in range(KT)]
            with ExitStack() as pA:
                wpool = pA.enter_context(tc.tile_pool(name="wpool", bufs=1))
                wq_t = [wpool.tile([P, D], F32, name=f"wq{g}") for g in range(KT)]
                wk_t = [wpool.tile([P, D], F32, name=f"wk{g}") for g in range(KT)]
                for g in range(KT):
                    nc.sync.dma_start(wq_t[g][:], wqst.ap()[g * P:(g + 1) * P, :])
                    nc.sync.dma_start(wk_t[g][:], wkt.ap()[g * P:(g + 1) * P, :])
                for e in range(KT):
                    for (lo, hi) in _chunks(D):
                        pm = ps0.tile([P, 512], F32, name=f"pm{e}_{lo}", tag="pm")
                        for g in range(KT):
                            nc.tensor.matmul(pm[:, :hi - lo], wq_t[g][:, e * P:(e + 1) * P],
                                             wk_t[g][:, lo:hi], start=(g == 0), stop=(g == KT - 1))
                        nc.scalar.copy(m_t[e][:, lo:hi], pm[:, :hi - lo])
            with ExitStack() as pB:
                xpool = pB.enter_context(tc.tile_pool(name="xpool", bufs=1))
                xi_t = [xpool.tile([P, B], F32, name=f"xi{e}") for e in range(KT)]
                for g in range(KT):
                    nc.sync.dma_start(xi_t[g][:], xit.ap()[g * P:(g + 1) * P, :])
                for d in range(KT):
                    for (lo, hi) in _chunks(B):
                        pa = ps0.tile([P, 512], F32, name=f"pa{d}_{lo}", tag="pm")
                        for e in range(KT):
                            nc.tensor.matmul(pa[:, :hi - lo], m_t[e][:, d * P:(d + 1) * P],
                                             xi_t[e][:, lo:hi], start=(e == 0), stop=(e == KT - 1))
                        a_f = xpool.tile([P, 512], F32, name=f"af{d}_{lo}", tag="af", bufs=2)
                        nc.scalar.copy(a_f[:, :hi - lo], pa[:, :hi - lo])
                        # hi = round_f32r(A); lo = round_f32r(A - hi)
                        nc.vector.tensor_copy(aith[d][:, lo:hi], a_f[:, :hi - lo])
                        al_f = xpool.tile([P, 512], F32, name=f"alf{d}_{lo}", tag="alf", bufs=2)
                        nc.vector.tensor_sub(al_f[:, :hi - lo], a_f[:, :hi - lo],
                                             aith[d][:, lo:hi].bitcast(F32))
                        nc.vector.tensor_copy(aitl[d][:, lo:hi], al_f[:, :hi - lo])

        # ---- flash over key super-blocks ----
        sp = ctx.enter_context(tc.tile_pool(name="sp", bufs=2 * sbn + 2))
        erp = sp
        xap = ctx.enter_context(tc.tile_pool(name="xap", bufs=3))
        xarp = ctx.enter_context(tc.tile_pool(name="xarp", bufs=sbn + 1))
        xtp = xap
        xthp = ctx.enter_context(tc.tile_pool(name="xthp", bufs=3))
        stat = ctx.enter_context(tc.tile_pool(name="stat", bufs=2))
        ps_s = ctx.enter_context(tc.tile_pool(name="ps_s", bufs=2, space="PSUM"))
        ps_o = ctx.enter_context(tc.tile_pool(name="ps_o", bufs=2, space="PSUM"))
        ps_t = ps_s

        # Manual logical-time slots, one-super-block lag: the XT split-casts
        # for block s run on DVE during block s-1's PE burst (prep slot), the
        # stats/exp chain for block s runs during block s+1's burst, and the
        # f32r out-matmuls for block s run as a clean burst after block s+1's
        # S-matmuls. PE never waits on the DVE chain, and same-dtype matmuls
        # stay contiguous (mode switches serialize LDWEIGHTS).
        omx = None
        for s in range(NSB):
            js = list(range(s * sbn, (s + 1) * sbn))
            base = 8.0 * s
            ssb, xar, xsplit = [], [], []
            if True:  # prep (emission order only; no sim-time slots)
                for j in js:
                    xt_t = xtp.tile([P, XAW], F32, name=f"xt{j}", tag="stg")
                    nc.sync.dma_start(xt_t[:, :D], xtj.ap()[j])
                    xth = xthp.tile([P, D], F32R, name=f"xth{j}", tag="xth")
                    nc.vector.tensor_copy(xth[:], xt_t[:, :D])
                    nc.vector.tensor_sub(xt_t[:, :D], xt_t[:, :D], xth[:].bitcast(F32))
                    xtl = xthp.tile([P, D], F32R, name=f"xtl{j}", tag="xtl")
                    nc.vector.tensor_copy(xtl[:], xt_t[:, :D])
                    xsplit.append((xth, xtl))

            if True:
                for idx, j in enumerate(js):
                    xth, xtl = xsplit[idx]
                    s_t = sp.tile([P, B], F32, name=f"s{j}", tag="s")
                    for (lo, hi) in _chunks(B):
                        pss = ps_s.tile([P, 512], F32, name=f"pss{j}_{lo}", tag="pss")
                        nmm = 3 * KT
                        i = 0
                        for k in range(KT):
                            kc = slice(k * P, (k + 1) * P)
                            for la, rb in ((xth, aith[k]), (xth, aitl[k]), (xtl, aith[k])):
                                nc.tensor.matmul(pss[:, :hi - lo], la[:, kc], rb[:, lo:hi],
                                                 start=(i == 0), stop=(i == nmm - 1))
                                i += 1
                        nc.scalar.copy(s_t[:, lo:hi], pss[:, :hi - lo])
                        # per-half running max: half-0's max finishes while PE
                        # still streams half-1, shortening the stats tail
                        nc.vector.tensor_max(gm[:, lo:hi], gm[:, lo:hi],
                                             s_t[:, lo:hi])
                    ssb.append(s_t)

            if True:
                # per-query-column running max (transpose-reduce gm chunks)
                nmx = stat.tile([P, MT], F32, name=f"nmx{s}", tag="nmx")
                corr = stat.tile([P, MT], F32, name=f"corr{s}", tag="corr")
                for c in range(MT):
                    pt = ps_t.tile([P, P], F32, name=f"pt{s}_{c}", tag="pss")
                    nc.tensor.transpose(pt[:], gm[:, c * P:(c + 1) * P], ident[:])
                    nc.vector.reduce_max(nmx[:, c:c + 1], pt[:], axis=AXX)
                if omx is None:
                    nc.vector.memset(corr[:], 0.0)
                else:
                    dmx = stat.tile([P, MT], F32, name=f"dmx{s}", tag="dmx")
                    nc.vector.tensor_sub(dmx[:], omx[:], nmx[:])
                    nc.scalar.activation(corr[:], dmx[:], EXP)
                omx = nmx

                # broadcast nmx (query-major) -> mxbc [P, B] (key-major free)
                ptb = ps_t.tile([P, P], F32, name=f"ptb{s}", tag="pss")
                nc.tensor.transpose(ptb[:MT, :], nmx[:], ident[:])
                mtmp = stat.tile([MT, P], F32, name=f"mtmp{s}", tag="mtmp")
                nc.scalar.copy(mtmp[:], ptb[:MT, :])
                mrow = stat.tile([1, B], F32, name=f"mrow{s}", tag="mrow", bufs=1)
                nc.sync.dma_start(mrow[:].rearrange("a (b c) -> a b c", b=MT), mtmp[:])
                nc.gpsimd.partition_broadcast(mxbc[:], mrow[:])

            if True:
                # E = exp(S - max) in fp32 in place, then DVE-cast to f32r
                # (the cast must be the f32r memory's only writer)
                ers = []
                for idx, s_t in enumerate(ssb):
                    nc.vector.tensor_sub(s_t[:], s_t[:], mxbc[:])
                    nc.scalar.activation(s_t[:], s_t[:], EXP)
                    er_t = erp.tile([P, B], F32R, name=f"er{s}_{idx}", tag="s")
                    nc.vector.tensor_copy(er_t[:], s_t[:])
                    ers.append(er_t)

            if True:
                for idx, j in enumerate(js):
                    xa_t = xap.tile([P, XAW], F32, name=f"xa{j}", tag="stg")
                    nc.sync.dma_start(xa_t[:], xa.ap()[j * P:(j + 1) * P, :])
                    xar_t = xarp.tile([P, XAW], F32R, name=f"xar{j}", tag="xar")
                    nc.gpsimd.tensor_copy(xar_t[:], xa_t[:])
                    xar.append(xar_t)

            if True:
                # out accumulation: acc = acc*corr + E^T @ X_aug (f32r burst);
                # 3 matmul streams into bank-aligned slices of one PSUM tile,
                # then a single fused rescale-accumulate per query tile
                for t in range(MT):
                    po = ps_o.tile([P, XAW], F32, name=f"po{s}_{t}", tag="po")
                    # j outer so the 3 column chunks reuse one stationary
                    # operand back-to-back (LDWEIGHTS locality); each chunk's
                    # PSUM accumulation group still spans idx 0..sbn-1
                    for idx in range(sbn):
                        er = ers[idx][:]
                        for (lo, hi) in _chunks(XAW):
                            nc.tensor.matmul(po[:, lo:hi], er[:, t * P:(t + 1) * P],
                                             xar[idx][:, lo:hi], start=(idx == 0), stop=(idx == sbn - 1))
                    nc.vector.scalar_tensor_tensor(acc[t][:], acc[t][:],
                                                   corr[:, t:t + 1], po[:],
                                                   op0=ALU.mult, op1=ALU.add)

        # ---- finalize: divide by the ones-column sums, write out ----
        if True:
            for t in range(MT):
                rc = stat.tile([P, 1], F32, name=f"rc{t}", tag="rc")
                nc.vector.reciprocal(rc[:], acc[t][:, D:D + 1])
                nc.vector.tensor_scalar_mul(acc[t][:, 0:D], acc[t][:, 0:D], rc[:])
                nc.sync.dma_start(out.ap()[t * P:(t + 1) * P, :], acc[t][:, 0:D])

    nc.compile()
    return nc


def prep_inputs(X, Wq, Wk, S, D, n_cores, aug=AUG):
    B = S // n_cores
    NT = S // P
    KT = D // P
    X = np.ascontiguousarray(X, np.float32)
    scale = np.float32(1.0 / np.sqrt(D))
    xtj = np.ascontiguousarray(
        X.reshape(NT, P, KT, P).transpose(0, 3, 2, 1).reshape(NT, P, D))
    xa = np.zeros((S, D + aug), np.float32)
    xa[:, :D] = X
    xa[:, D] = 1.0
    wqst = np.ascontiguousarray((np.asarray(Wq, np.float32) * scale).T)
    wkt = np.ascontiguousarray(np.asarray(Wk, np.float32).T)
    xt = X.T
    in_maps = []
    for i in range(n_cores):
        in_maps.append({
            "xtj": xtj, "xa": xa, "wqst": wqst, "wkt": wkt,
            "xit": np.ascontiguousarray(xt[:, i * B:(i + 1) * B]),
        })
    return in_maps


_CACHE = {}


def _get_kernel(S, D, B, sbn):
    key = (S, D, B, sbn)
    if key not in _CACHE:
        _CACHE[key] = build_core_kernel(S, D, B, sbn=sbn)
    return _CACHE[key]


def kernel(inputs, weight_query, weight_key):
    S, D = inputs.shape
    assert (S, D) == (SEQ, DIM)
    B = S // NCORES
    nc = _get_kernel(S, D, B, SBN)
    in_maps = prep_inputs(inputs, weight_query, weight_key, S, D, NCORES)
    res = run_bass_kernel_spmd(nc, in_maps, core_ids=list(range(NCORES)))
    return np.concatenate([res.results[i]["out"] for i in range(NCORES)], axis=0)


if __name__ == "__main__":
    rng = np.random.default_rng(0)
    X = rng.standard_normal((SEQ, DIM), dtype=np.float32)
    Wq = rng.standard_normal((DIM, DIM), dtype=np.float32)
    Wk = rng.standard_normal((DIM, DIM), dtype=np.float32)
    out = kernel(X, Wq, Wk)
    print(out.shape, out.dtype)

